# revision 65
# baseline (speedup 1.0000x reference)
"""Trainium2 Bass kernel v3: single-head causal attention (B=4, T=4096, C=2048, H=128).

    q = x @ Wq; k = x @ Wk; v = x @ Wv        (per batch element)
    out = softmax(causal(q k^T * C**-0.5)) @ v

Sharding: two cores per batch element (8 cores, B=4); core parity p owns
q row-blocks p, p+2, ... (128-row-block parity split balances causal work).
v3 removes the duplicated k/v projection of v2: each core projects q/k/v
only for its OWN parity rows (half the x DMA, half the k/v matmuls), then
the pair exchanges k^T and natural-layout v via per-group pair-wise
AllGather collectives (HBM bounce buffers). The SBUF k/v layout is
RANK-ordered (rank 0 = even-parity core first), so the program is
identical on both cores; the parity-dependence of the causal structure
lives entirely in the mask DATA: p=0 -> [tri, zeros], p=1 -> [ones, tri]
for the two diagonal-band slots (dd == 2c and dd == 2c+1).

Pipeline per 512-column group g: project kT/v/qT for own columns of g
(q/k as fp8 DoubleRow matmuls — see FP8_QK below — v directly in natural
[k,H] blocks by swapping the matmul operands: out[k,H] = x_chunk^T @
Wv_chunk, no DMA transpose), copy k|v to a bounce tile, AllGather with
the partner (~4us, measured nearly free), read both rank halves back into
kTf/vav on the sync HWDGE ring. Attention lags projections by two groups
so each exchange hides under ~20us of matmuls. Per-group x tiles in a
bufs=2 pool throttle the x stream so it cannot monopolize the DMA engines
while an exchange is in flight. Scores-transposed attention, exp on the
scalar engine, denominator via an appended ones-column in v — as in v2.
"""

import numpy as np
import ml_dtypes

B, T, C, H = 4, 4096, 2048, 128
NCORES = 8
TQ = T // 2              # per-core q rows == per-core own k rows
NCC = C // 128           # 16 contraction chunks
NSB = TQ // 512          # 4 q superblocks of 512 rows per core
NG = TQ // 512           # 4 own-column projection groups
SCALE = float(C) ** -0.5
BF16 = ml_dtypes.bfloat16
FP8 = ml_dtypes.float8_e4m3fn
VF = 144                 # v-block stride in vav (ones column at 128)

# fp8 q/k projections (DoubleRow, 2x PE throughput on the 512-wide moving
# dim). x additionally ships as x8 = fp8(x) (4MB) next to the bf16 x used
# by the v projection; Wq/Wk are pre-scaled by 32 before the e4m3 cast
# (keeps them out of the subnormal range) and the 2^-10 q*k descale folds
# into the exp scale. v stays bf16 (its error hits the output directly).
# DoubleRow with a 128-wide moving dim (natural-layout v) measured SLOWER
# than bf16 on hardware — weight-load bound — so only q/k use it.
FP8_QK = True
WS = 32.0                # weight pre-scale
EXP_SCALE = SCALE / (WS * WS)

# Diagonal-band block classification in RANK layout, (dd, c) with
# dd = 2*(d - 4s) + rank for k block (rank, d), c = q chunk index:
#   dd <  2c   : fully allowed for both parities — no mask
#   dd == 2c   : slot 0 — tri for p=0 (G==Q), ones for p=1 (G==Q-1)
#   dd == 2c+1 : slot 1 — zeros for p=0 (G==Q+1, dead), tri for p=1 (G==Q)
#   dd >  2c+1 : dead for both parities — AV matmul and mask skipped
NSLOT = 2

# Profiling aid only (tlprof.py): replace the pair AllGather with two local
# DRAM copies so the single-core TimelineSim (no collective support, and a
# wildly pessimistic collective cost model) sees a representative schedule.
# Numerically wrong — never set on the graded path.
SIM_NO_CC = False

_cached = {}


def _emit(nc, tc, tile, mybir, xT, wq, wk, wv, msk, out, rep, row_base,
          x8T=None, xr8T=None, w8q=None, w8k=None, w8v=None, w8vr=None):
    f32 = mybir.dt.float32
    bf16 = mybir.dt.bfloat16
    f8 = mybir.dt.float8e4
    AF = mybir.ActivationFunctionType
    DR = mybir.MatmulPerfMode.DoubleRow

    with tc.tile_pool(name=f"sb{rep}", bufs=1) as sb, \
         tc.tile_pool(name=f"sbs{rep}", bufs=4) as sbs, \
         tc.tile_pool(name=f"st{rep}", bufs=2) as st_pool, \
         tc.tile_pool(name=f"xp{rep}", bufs=2) as xp_pool, \
         tc.tile_pool(name=f"p_sb{rep}", bufs=4) as p_pool, \
         tc.tile_pool(name=f"o_sb{rep}", bufs=2) as o_pool, \
         tc.tile_pool(name=f"dram{rep}", bufs=1, space="DRAM") as dram, \
         tc.tile_pool(name=f"ps_w{rep}", bufs=1, space="PSUM") as ps_proj, \
         tc.tile_pool(name=f"ps_s{rep}", bufs=2, space="PSUM") as pssp, \
         tc.tile_pool(name=f"ps_o{rep}", bufs=1, space="PSUM") as psop:

        # ---- resident loads (scalar-engine HWDGE ring) ----------
        w_names = ("wv",) if FP8_QK else ("wq", "wk", "wv")
        w_sb = {name: sb.tile([128, NCC * H], bf16, tag=name, name=f"w_{name}{rep}")
                for name in w_names}
        if FP8_QK:
            w8_sb = {name: sb.tile([128, NCC * H], f8, tag=f"w8{name}",
                                   name=f"w8_{name}{rep}")
                     for name in ("wq", "wk")}
        mask_sb = sb.tile([128, NSLOT * 128], bf16)
        # Per-group x tiles in a bufs=2 pool: group g+2's load carries a WAR
        # dependency on proj(g)'s last read, which throttles the x stream so
        # it never monopolizes the DMA engines while an exchange is in
        # flight (loading all 8MB up front starves the first readback).
        x_tiles = {}
        x_views = {}

        def load_w(name, h, c0=0, c1=NCC):
            nc.scalar.dma_start(
                w_sb[name][:].rearrange("p (n h) -> p n h", h=H)[:, c0:c1, :],
                h.ap().rearrange("(n p) h -> p n h", p=128)[:, c0:c1, :])

        def load_w8(name, h, c0=0, c1=NCC):
            nc.scalar.dma_start(
                w8_sb[name][:].rearrange("p (n h) -> p n h", h=H)[:, c0:c1, :],
                h.ap().rearrange("(n p) h -> p n h", p=128)[:, c0:c1, :])

        def load_x(kind, src, dt, g, c0, c1):
            if (kind, g) not in x_tiles:
                x_tiles[kind, g] = xp_pool.tile([128, NCC * 512], dt,
                                                tag=kind,
                                                name=f"{kind}{rep}_{g}")
                x_views[kind, g] = x_tiles[kind, g][:].rearrange(
                    "p (n t) -> p n t", t=512)
            base = 512 * g
            sdr = src.ap().rearrange("(n p) t -> p n t", p=128)
            nc.scalar.dma_start(x_views[kind, g][:, c0:c1, :],
                                sdr[:, c0:c1, base:base + 512])

        if FP8_QK:
            load_w8("wk", w8k, 0, 4)
            load_x("x8", x8T, f8, 0, 0, 2)
            load_w8("wk", w8k, 4, NCC)
            load_x("x8", x8T, f8, 0, 2, 4)
            load_x("x8", x8T, f8, 0, 4, NCC)
            load_w8("wq", w8q)
            load_w("wv", wv)
            load_x("x", xT, bf16, 0, 0, 4)
            load_x("x", xT, bf16, 0, 4, NCC)
            nc.scalar.dma_start(mask_sb[:], msk.ap())
            for g in range(1, NG):
                load_x("x8", x8T, f8, g, 0, NCC)
                load_x("x", xT, bf16, g, 0, 8)
                load_x("x", xT, bf16, g, 8, NCC)
        else:
            load_w("wk", wk, 0, 8)
            load_x("x", xT, bf16, 0, 0, 2)
            load_w("wk", wk, 8, NCC)
            load_x("x", xT, bf16, 0, 2, 4)
            load_x("x", xT, bf16, 0, 4, 8)
            load_x("x", xT, bf16, 0, 8, NCC)
            load_w("wv", wv)
            load_w("wq", wq)
            nc.scalar.dma_start(mask_sb[:], msk.ap())
            for g in range(1, NG):
                load_x("x", xT, bf16, g, 0, 8)
                load_x("x", xT, bf16, g, 8, NCC)

        def wchunk(name, cc):
            return w_sb[name][:, cc * H:(cc + 1) * H]

        def w8pair(name, c2):
            return w8_sb[name][:].rearrange(
                "p (n h) -> p n h", h=H)[:, 2 * c2:2 * c2 + 2, :]

        kTf_sb = sb.tile([128, T], bf16)
        qT_sb = sb.tile([128, TQ], bf16)
        v_all = sb.tile([128, 32 * VF], bf16)
        vav = v_all[:].rearrange("p (n f) -> p n f", f=VF)
        nc.gpsimd.memset(vav[:, :, 128:129], 1.0)

        # ---- per-group projection + pair exchange ---------------
        def proj_group(g):
            base = 512 * g

            stage = st_pool.tile([128, 1024], bf16, tag="stage",
                                 name=f"stage{rep}_{g}")

            def proj_qk(wname, ps):
                if FP8_QK:
                    xv8 = x_views["x8", g]
                    for c2 in range(NCC // 2):
                        nc.tensor.matmul(ps[:], w8pair(wname, c2),
                                         xv8[:, 2 * c2:2 * c2 + 2, :],
                                         start=(c2 == 0),
                                         stop=(c2 == NCC // 2 - 1),
                                         perf_mode=DR)
                else:
                    xv = x_views["x", g]
                    for cc in range(NCC):
                        nc.tensor.matmul(ps[:], wchunk(wname, cc),
                                         xv[:, cc, :],
                                         start=(cc == 0),
                                         stop=(cc == NCC - 1))

            # q and k share one PSUM bank (tag "qk"): by the time the q
            # matmuls run (after the 3.4us v projection), the k copy has
            # long drained, so the WAR costs nothing and frees a bank for
            # the two-bank score tiles below.
            psk = ps_proj.tile([128, 512], f32, tag="qk",
                               name=f"psk{rep}_{g}")
            proj_qk("wk", psk)
            nc.vector.tensor_copy(stage[:, 0:512], psk[:])

            # v in natural [k,H] layout: out = x_chunk^T @ Wv_chunk.
            # All 4 row-blocks (and, for fp8, all 3 correction terms)
            # accumulate in one PSUM bank; the first matmul's start clears
            # the whole bank (emitted after the previous group's copy has
            # read it — pool WAR dep).
            psv = ps_proj.tile([128, 512], f32, tag="v",
                               name=f"psv{rep}_{g}")
            xv = x_views["x", g]
            for c in range(4):
                for cc in range(NCC):
                    nc.tensor.matmul(psv[:, 128 * c:128 * (c + 1)],
                                     xv[:, cc, 128 * c:128 * (c + 1)],
                                     wchunk("wv", cc),
                                     start=(c == 0 and cc == 0),
                                     stop=(c == 3 and cc == NCC - 1),
                                     skip_group_check=True)
            nc.vector.tensor_copy(stage[:, 512:1024], psv[:])

            psq = ps_proj.tile([128, 512], f32, tag="qk",
                               name=f"psq{rep}_{g}")
            proj_qk("wq", psq)
            nc.vector.tensor_copy(qT_sb[:, base:base + 512], psq[:])

            # bounce out (gpsimd), pair AllGather (gpsimd), read both rank
            # halves back on the sync HWDGE ring (nothing else queues there
            # until the attention output stores, which ride vector).
            cc_in = dram.tile([128, 1024], bf16, name=f"cci{rep}_{g}")
            cc_out = dram.tile([256, 1024], bf16, name=f"cco{rep}_{g}")
            # bounce write on the sync HWDGE ring (not Pool SWDGE, which
            # would execute the copy on the gpsimd engine itself); the
            # collective's data dependency on it is cross-engine.
            nc.sync.dma_start(cc_in[:], stage[:])
            if SIM_NO_CC:
                nc.gpsimd.dma_start(cc_out[0:128, :], cc_in[:])
                nc.gpsimd.dma_start(cc_out[128:256, :], cc_in[:])
            else:
                nc.gpsimd.collective_compute(
                    "AllGather", mybir.AluOpType.bypass,
                    replica_groups=[[0, 1], [2, 3], [4, 5], [6, 7]],
                    ins=[cc_in.opt()], outs=[cc_out.opt()])
            for r in range(2):
                rows = cc_out[128 * r:128 * (r + 1), 0:512]
                nc.sync.dma_start(
                    kTf_sb[:, TQ * r + base:TQ * r + base + 512], rows)
                vsrc = cc_out[128 * r:128 * (r + 1), 512:1024]
                nc.sync.dma_start(
                    vav[:, 16 * r + 4 * g:16 * r + 4 * (g + 1), 0:128],
                    vsrc.rearrange("p (n f) -> p n f", f=128))

        def kt_blk(j):
            base = TQ * (j % 2) + 128 * (j // 2)
            return kTf_sb[:, base:base + 128]

        def v_blk(j):
            return vav[:, (j % 2) * 16 + j // 2, 0:129]

        # ---- attention (pipelined with projections) -------------
        def o_chunk(tiles, c):
            t = tiles[c // 2]
            off = 129 * (c % 2)
            return t[:, off:off + 129]

        state = {"pending": []}  # deque of (s, u, npair, P, o_tiles)

        def kept(s, u, half, c):
            if u < 4 * s:
                return True
            return 2 * (u - 4 * s) + half <= 2 * c + 1

        def last_kept(s, bank):
            npair = 4 * s + 4
            return max((u, half, c) for u in range(npair) for half in (0, 1)
                       for c in (2 * bank, 2 * bank + 1) if kept(s, u, half, c))

        def emit_av(p):
            s, u, npair, P, o_tiles = p
            stops = (last_kept(s, 0), last_kept(s, 1))
            for half in range(2):
                j = 2 * u + half
                for c in range(4):
                    if not kept(s, u, half, c):
                        continue
                    nc.tensor.matmul(
                        o_chunk(o_tiles, c),
                        P[:, 512 * half + 128 * c:
                           512 * half + 128 * (c + 1)],
                        v_blk(j),
                        # start clears the whole PSUM bank: only the
                        # first chunk written to each bank may set it
                        start=(u == 0 and half == 0 and c % 2 == 0),
                        stop=((u, half, c) == stops[c // 2]),
                        skip_group_check=True)
            if u == npair - 1:
                # superblock finished: normalize + store. The final
                # superblock stores per bank-pair so the first half's DMA
                # overlaps the second half's normalize (shortens the tail).
                osb = o_pool.tile([128, 4 * H], f32, tag="osb")
                r0 = row_base + 512 * s
                odr = out.ap()[r0:r0 + 512, :].rearrange(
                    "(n p) h -> p n h", p=128)
                osv = osb[:].rearrange("p (n h) -> p n h", h=H)
                splits = ((0, 2), (2, 4)) if s == NSB - 1 else ((0, 4),)
                for lo, hi in splits:
                    for c in range(lo, hi):
                        po = o_chunk(o_tiles, c)
                        rec = sbs.tile([128, 1], f32, tag="rec")
                        nc.vector.reciprocal(rec[:], po[:, 128:129])
                        nc.vector.tensor_scalar_mul(
                            osb[:, c * H:(c + 1) * H], po[:, 0:128], rec[:])
                    nc.scalar.dma_start(odr[:, lo:hi, :], osv[:, lo:hi, :])

        o_state = {}

        def attn_block(s, u0, u1):
            if u0 == 0:
                o_state[s] = (psop.tile([128, 258], f32, tag="oa",
                                        name=f"oa{rep}_{s}"),
                              psop.tile([128, 258], f32, tag="ob",
                                        name=f"ob{rep}_{s}"))
            o_tiles = o_state[s]
            npair = 4 * s + 4
            es = EXP_SCALE if FP8_QK else SCALE
            for u in range(u0, u1):
                P = p_pool.tile([128, 1024], bf16, tag="P",
                                name=f"P{rep}_{s}_{u}")
                # Both halves' scores land in one two-bank PSUM tile (each
                # half's matmul start clears only its own bank); off-band
                # units then take a single fused 1024-wide exp.
                band = u >= 4 * s
                pss = pssp.tile([128, 1024], f32, tag="pss",
                                name=f"pss{rep}_{s}_{u}")
                for half in range(2):
                    j = 2 * u + half
                    # kept q-chunks form a suffix (c >= dd//2): narrow the
                    # S matmul and exp to it — dead chunks are never read
                    # (their AV matmuls are skipped below).
                    c0 = (2 * (u - 4 * s) + half) // 2 if band else 0
                    lo = 512 * half + 128 * c0
                    nc.tensor.matmul(
                        pss[:, lo:512 * (half + 1)], kt_blk(j),
                        qT_sb[:, 512 * s + 128 * c0:512 * (s + 1)],
                        start=True, stop=True, skip_group_check=True)
                    if band:
                        nc.scalar.activation(P[:, lo:512 * (half + 1)],
                                             pss[:, lo:512 * (half + 1)],
                                             AF.Exp, scale=es)
                        dd = 2 * (u - 4 * s) + half
                        for c in range(4):
                            if dd in (2 * c, 2 * c + 1):
                                si = dd - 2 * c
                                pc = P[:, 512 * half + 128 * c:
                                       512 * half + 128 * (c + 1)]
                                nc.vector.tensor_mul(
                                    pc, pc,
                                    mask_sb[:, 128 * si:128 * (si + 1)])
                if not band:
                    nc.scalar.activation(P[:], pss[:], AF.Exp, scale=es)
                if len(state["pending"]) == 2:
                    emit_av(state["pending"].pop(0))
                state["pending"].append((s, u, npair, P, o_tiles))

        # attention lags projections by two groups: each group's exchange
        # (stage copy -> bounce -> AllGather -> readback) gets ~20us of
        # projection cover before its superblock needs the data.
        proj_group(0)
        proj_group(1)
        proj_group(2)
        attn_block(0, 0, 4)
        proj_group(3)
        attn_block(1, 0, 8)
        attn_block(2, 0, 12)
        attn_block(3, 0, 16)
        for p in state["pending"]:
            emit_av(p)
        state["pending"] = []


def _build_nc(n_repeat=1):
    # n_repeat > 1 builds an in-NEFF benchmarking loop: the kernel body is
    # emitted n_repeat times, each writing its own live row-range of
    # the output (a fully shared output would let the compiler prune
    # overwritten repeats).
    import concourse.bacc as bacc
    import concourse.mybir as mybir
    from concourse import tile

    f32 = mybir.dt.float32
    bf16 = mybir.dt.bfloat16

    nc = bacc.Bacc("TRN2", target_bir_lowering=False, debug=False,
                   num_devices=NCORES)

    f8 = mybir.dt.float8e4
    xT = nc.declare_dram_parameter("xT", [C, TQ], bf16, isOutput=False)
    wv = nc.declare_dram_parameter("Wv", [C, H], bf16, isOutput=False)
    if FP8_QK:
        wq = wk = None
        x8T = nc.declare_dram_parameter("x8T", [C, TQ], f8, isOutput=False)
        w8q = nc.declare_dram_parameter("W8q", [C, H], f8, isOutput=False)
        w8k = nc.declare_dram_parameter("W8k", [C, H], f8, isOutput=False)
    else:
        x8T = w8q = w8k = None
        wq = nc.declare_dram_parameter("Wq", [C, H], bf16, isOutput=False)
        wk = nc.declare_dram_parameter("Wk", [C, H], bf16, isOutput=False)
    msk = nc.declare_dram_parameter("masks", [128, NSLOT * 128], bf16,
                                    isOutput=False)
    out = nc.declare_dram_parameter("out", [n_repeat * TQ, H], f32,
                                    isOutput=True)

    with tile.TileContext(nc) as tc:
        for rep in range(n_repeat):
            _emit(nc, tc, tile, mybir, xT, wq, wk, wv, msk, out, rep,
                  rep * TQ, x8T=x8T, w8q=w8q, w8k=w8k)

    nc.finalize()
    return nc


def _build_masks(p):
    # Rank layout (see NSLOT comment): slot 0 covers dd == 2c (k block
    # G == Q - p), slot 1 covers dd == 2c+1 (G == Q + 1 - 2p).
    kk = np.arange(128)[:, None]
    tt = np.arange(128)[None, :]
    tri = (kk <= tt).astype(np.float32)
    if p == 0:
        slots = [tri, np.zeros((128, 128), np.float32)]
    else:
        slots = [np.ones((128, 128), np.float32), tri]
    return np.ascontiguousarray(
        np.concatenate(slots, axis=1).astype(BF16))


def _get_nc():
    if "nc" not in _cached:
        _cached["nc"] = _build_nc()
        _cached["masks"] = {p: _build_masks(p) for p in (0, 1)}
    return _cached["nc"]


def _prep_in_maps(x, Wq, Wk, Wv):
    _get_nc()
    in_maps = []
    if FP8_QK:
        wcom = {"Wv": np.ascontiguousarray(np.asarray(Wv).astype(BF16)),
                "W8q": np.ascontiguousarray(
                    (np.asarray(Wq, np.float32) * WS).astype(FP8)),
                "W8k": np.ascontiguousarray(
                    (np.asarray(Wk, np.float32) * WS).astype(FP8))}
    else:
        wcom = {n: np.ascontiguousarray(np.asarray(w).astype(BF16))
                for n, w in (("Wq", Wq), ("Wk", Wk), ("Wv", Wv))}
    xb16 = [np.asarray(x[b]).astype(BF16).reshape(T // 128, 128, C)
            for b in range(B)]
    if FP8_QK:
        xb8 = [np.asarray(x[b], np.float32).astype(FP8)
               .reshape(T // 128, 128, C) for b in range(B)]
    for c in range(NCORES):
        b, p = divmod(c, 2)
        m = {"xT": np.ascontiguousarray(xb16[b][p::2].reshape(TQ, C).T),
             "masks": _cached["masks"][p], **wcom}
        if FP8_QK:
            m["x8T"] = np.ascontiguousarray(xb8[b][p::2].reshape(TQ, C).T)
        in_maps.append(m)
    return in_maps


def _gather_out(results):
    out = np.empty((B, T, H), np.float32)
    for c in range(NCORES):
        b, p = divmod(c, 2)
        out[b].reshape(T // 128, 128, H)[p::2] = \
            results[c]["out"].reshape(TQ // 128, 128, H)
    return out


def kernel(x, Wq, Wk, Wv):
    from concourse.bass_utils import run_bass_kernel_spmd

    nc = _get_nc()
    in_maps = _prep_in_maps(x, Wq, Wk, Wv)
    res = run_bass_kernel_spmd(nc, in_maps, list(range(NCORES)))
    return _gather_out(res.results)
